# revision 39
# baseline (speedup 1.0000x reference)
"""Block-sparse (block-diagonal, BLOCK=64) multi-head attention for 8 Trainium2 cores.

Sharding: the B*S = 4096 token rows are split into 8 contiguous slices of 512
tokens (attention is block-diagonal with 64-token blocks, so slices at
512-token boundaries are fully independent). Each core runs the whole
projections + attention + output projection for its 512 tokens; weights are
replicated. No collectives; host concatenates the per-core outputs.

Schedule (from iterative trace analysis; ~114us -> ~99us):
  - exact algebraic simplifications: bk dropped entirely (a per-query-row
    constant added to in-block scores is softmax-invariant), and bv folded
    into bo on the host (attention rows sum to 1, so +bv passes through
    attention: y = attn(..)Wo + (bv Wo + bo)).
  - DMA: two queues (sync HWDGE + gpsimd SWDGE) ordered by first use, with
    the first Q-proj matmul's operands (xq quarter / wq0 / bq) heading
    DIFFERENT queues. First completion sems fire ~13us (7us TileContext
    prologue + ~5.5us queue-pipeline ramp, size-independent); ~70 junk
    warmup matmuls bridge that window and keep the PE HAM clock gate warm.
  - attention processes TWO head pairs per super-iteration: K=64 score
    matmuls (one per head, each into a full [128,128] psum tile -- K<128
    matmuls mis-lower on HW for any subrange/offset psum out), ONE
    [128,128] Exp per head (vs 4 quadrant exps -- scalar was the hidden
    serializer), gpsimd memsets zero the exp(garbage) cross-block
    quadrants, then K=128 col-split matmuls compute replicated row sums +
    O^T into one [128,256] psum tile. The loop is software-pipelined one
    super-iteration deep: each SI's rowsum/recip/O^T/mul back half is
    emitted during the NEXT SI, with the 8 dense V/Y-projection matmuls
    in between, so the PE never waits on the exp+memset chain.
  - y output DMA'd as fp16 (halves output bytes; well within tolerance).
  - y projection for chunk c runs interleaved inside chunk c+1 (one PSUM
    accumulator at a time); the last chunk's y parts are squeezed into its
    own iterations so only the m6,m7 accumulations + adds + 2 output DMAs
    trail the final softmax mul.

Compute dtype: fp16 operands with fp32 PSUM accumulation everywhere.
"""

import sys

sys.path.insert(0, "/opt/trn_rl_repo")

import numpy as np

N_CORES = 8
B, S, D = 2, 2048, 1024
H, DK = 16, 64
T = (B * S) // N_CORES      # 512 tokens per core
P = 128
KO = D // P                 # 8 contraction tiles
MO = D // P                 # 8 d_out tiles
NC_CHUNKS = T // P          # 4 token chunks per core
HP = H // 2                 # 8 head pairs

WARMUP = 100

_cache = {}


def _build_program(compute):
    import concourse.tile as tile
    from concourse import bacc, mybir

    f32 = mybir.dt.float32
    dtc = {"f32": f32, "f16": mybir.dt.float16, "bf16": mybir.dt.bfloat16}[compute]

    nc = bacc.Bacc("TRN2", target_bir_lowering=False, debug=False)

    xq_d = nc.dram_tensor("xq", [P, KO, T], dtc, kind="ExternalInput").ap()
    xk_d = nc.dram_tensor("xk", [P, KO, T], dtc, kind="ExternalInput").ap()
    xv_d = nc.dram_tensor("xv", [P, KO, T], dtc, kind="ExternalInput").ap()
    wq_d = nc.dram_tensor("wq", [MO, P, KO, P], dtc, kind="ExternalInput").ap()
    wk_d = nc.dram_tensor("wk", [MO, P, KO, P], dtc, kind="ExternalInput").ap()
    wv_d = nc.dram_tensor("wv", [D, D], dtc, kind="ExternalInput").ap()
    wo_d = nc.dram_tensor("wo", [D, D], dtc, kind="ExternalInput").ap()
    bq_d = nc.dram_tensor("bq", [P, MO], f32, kind="ExternalInput").ap()
    bo_d = nc.dram_tensor("bo", [D], dtc, kind="ExternalInput").ap()
    y_d = nc.dram_tensor("y", [T, D], dtc, kind="ExternalOutput").ap()

    with tile.TileContext(nc) as tc:
        with (
            tc.tile_pool(name="singles", bufs=1) as singles,
            tc.tile_pool(name="p2", bufs=4) as p2_pool,
            tc.tile_pool(name="rec", bufs=2) as rec_pool,
            tc.tile_pool(name="ystage", bufs=3) as y_pool,
            tc.tile_pool(name="psproj", bufs=2, space="PSUM") as psproj,
            tc.tile_pool(name="pss", bufs=4, space="PSUM") as pss_pool,
            tc.tile_pool(name="psro", bufs=2, space="PSUM") as psro_pool,
        ):
            # ---- persistent SBUF tensors ----
            xq_sb = singles.tile([P, KO, T], dtc, tag="xq")
            xk_sb = singles.tile([P, KO, T], dtc, tag="xk")
            xv_sb = singles.tile([P, KO, T], dtc, tag="xv")
            wq_t = [singles.tile([P, KO, P], dtc, tag=f"wq{i}", name=f"wq{i}") for i in range(MO)]
            wk_t = [singles.tile([P, KO, P], dtc, tag=f"wk{i}", name=f"wk{i}") for i in range(MO)]
            wv_t = [singles.tile([P, D], dtc, tag=f"wv{i}", name=f"wv{i}") for i in range(KO)]
            wo_t = [singles.tile([P, D], dtc, tag=f"wo{i}", name=f"wo{i}") for i in range(KO)]
            qT_sb = singles.tile([P, MO, T], dtc, tag="qT")
            kT_sb = singles.tile([P, MO, T], dtc, tag="kT")
            v_sb = singles.tile([P, NC_CHUNKS, D], dtc, tag="v")
            oT_sb = singles.tile([P, MO, T], dtc, tag="oT")
            bq_sb = singles.tile([P, MO], f32, tag="bq")
            bo_sb = singles.tile([P, D], dtc, tag="bo")
            ones_sb = singles.tile([P, 64], dtc, tag="ones")

            # PE warm-up: junk matmuls while the DMA lead-in runs, sized to
            # end right as the first projection operands arrive
            nc.vector.memset(ones_sb[:], 1.0)
            ps_w = psro_pool.tile([64, 64], f32, tag="psro", name="warmup")
            for _ in range(WARMUP):
                nc.tensor.matmul(ps_w[:], ones_sb[:, 0:64], ones_sb[:, 0:64],
                                 start=True, stop=True)

            # ---- input DMAs, two queues, ordered so the first Q-proj
            # matmul's operands (xq quarter 0 + wq0) head different queues
            def xpart(sb, dr, a, b):
                return (sb[:, a:b, :].rearrange("p k t -> p (k t)"),
                        dr[:, a:b, :].rearrange("p k t -> p (k t)"))

            def wtile(t, dr):
                return (t[:].rearrange("p k c -> p (k c)"), dr)

            sq = [
                xpart(xq_sb, xq_d, 0, 2), xpart(xq_sb, xq_d, 2, 4),
                wtile(wq_t[1], wq_d[1].rearrange("p k c -> p (k c)")),
                wtile(wq_t[3], wq_d[3].rearrange("p k c -> p (k c)")),
                wtile(wq_t[5], wq_d[5].rearrange("p k c -> p (k c)")),
                wtile(wq_t[7], wq_d[7].rearrange("p k c -> p (k c)")),
                xpart(xk_sb, xk_d, 0, 4),
                wtile(wk_t[1], wk_d[1].rearrange("p k c -> p (k c)")),
                wtile(wk_t[3], wk_d[3].rearrange("p k c -> p (k c)")),
                wtile(wk_t[5], wk_d[5].rearrange("p k c -> p (k c)")),
                wtile(wk_t[7], wk_d[7].rearrange("p k c -> p (k c)")),
                xpart(xv_sb, xv_d, 0, 4),
                (wv_t[0][:], wv_d[0:P, :]), (wv_t[2][:], wv_d[2 * P:3 * P, :]),
                (wv_t[4][:], wv_d[4 * P:5 * P, :]), (wv_t[6][:], wv_d[6 * P:7 * P, :]),
                (wo_t[0][:], wo_d[0:P, :]), (wo_t[2][:], wo_d[2 * P:3 * P, :]),
                (wo_t[4][:], wo_d[4 * P:5 * P, :]), (wo_t[6][:], wo_d[6 * P:7 * P, :]),
            ]
            gq = [
                (bq_sb[:], bq_d[:]),
                wtile(wq_t[0], wq_d[0].rearrange("p k c -> p (k c)")),
                xpart(xq_sb, xq_d, 4, 8),
                wtile(wq_t[2], wq_d[2].rearrange("p k c -> p (k c)")),
                wtile(wq_t[4], wq_d[4].rearrange("p k c -> p (k c)")),
                wtile(wq_t[6], wq_d[6].rearrange("p k c -> p (k c)")),
                xpart(xk_sb, xk_d, 4, 8),
                wtile(wk_t[0], wk_d[0].rearrange("p k c -> p (k c)")),
                wtile(wk_t[2], wk_d[2].rearrange("p k c -> p (k c)")),
                wtile(wk_t[4], wk_d[4].rearrange("p k c -> p (k c)")),
                wtile(wk_t[6], wk_d[6].rearrange("p k c -> p (k c)")),
                xpart(xv_sb, xv_d, 4, 8),
                (wv_t[1][:], wv_d[P:2 * P, :]), (wv_t[3][:], wv_d[3 * P:4 * P, :]),
                (wv_t[5][:], wv_d[5 * P:6 * P, :]), (wv_t[7][:], wv_d[7 * P:8 * P, :]),
                (bo_sb[:], bo_d[None, :].to_broadcast([P, D])),
                (wo_t[1][:], wo_d[P:2 * P, :]), (wo_t[3][:], wo_d[3 * P:4 * P, :]),
                (wo_t[5][:], wo_d[5 * P:6 * P, :]), (wo_t[7][:], wo_d[7 * P:8 * P, :]),
            ]
            for dst, src in sq:
                nc.sync.dma_start(dst, src)
            for dst, src in gq:
                nc.gpsimd.dma_start(dst, src)

            # ---- Q projection (feature-major out, bias via scalar) ----
            # after m0/m1, more keep-warm junk: the PE idles there waiting
            # for the next weight tile anyway, and on slow-DMA runs the
            # idle would re-throttle the HAM clock gate for the whole
            # early QK phase (~3us penalty)
            for m in range(MO):
                ps = psproj.tile([P, T], f32, tag="psproj", name=f"psq{m}")
                for ko in range(KO):
                    nc.tensor.matmul(ps[:], wq_t[m][:, ko, :], xq_sb[:, ko, :],
                                     start=(ko == 0), stop=(ko == KO - 1))
                nc.scalar.activation(qT_sb[:, m, :], ps[:],
                                     mybir.ActivationFunctionType.Identity,
                                     bias=bq_sb[:, m:m + 1])
                for _ in range({0: 30, 1: 10}.get(m, 0)):
                    nc.tensor.matmul(ps_w[:], ones_sb[:, 0:64], ones_sb[:, 0:64],
                                     start=True, stop=True)
            # ---- K projection (no bias: softmax-invariant) ----
            for m in range(MO):
                ps = psproj.tile([P, T], f32, tag="psproj", name=f"psk{m}")
                for ko in range(KO):
                    nc.tensor.matmul(ps[:], wk_t[m][:, ko, :], xk_sb[:, ko, :],
                                     start=(ko == 0), stop=(ko == KO - 1))
                nc.vector.tensor_copy(kT_sb[:, m, :], ps[:])

            # ---- V / Y projection part helpers ----
            v_ps = [None] * NC_CHUNKS
            y_ps = [None] * NC_CHUNKS
            y_tail = [None]

            def v_proj_part(mt, part):
                # part 0..7: n half = part//4, contraction pair (2k, 2k+1)
                n, k2 = part // 4, 2 * (part % 4)
                if part % 4 == 0:
                    v_ps[mt] = psproj.tile([P, T], f32, tag="psproj", name=f"psv_{mt}_{n}")
                for ko in (k2, k2 + 1):
                    nc.tensor.matmul(v_ps[mt][:],
                                     xv_sb[:, ko, mt * P:(mt + 1) * P],
                                     wv_t[ko][:, n * T:(n + 1) * T],
                                     start=(part % 4 == 0 and ko == k2),
                                     stop=(part % 4 == 3 and ko == k2 + 1))
                if part % 4 == 3:
                    nc.vector.tensor_copy(v_sb[:, mt, n * T:(n + 1) * T], v_ps[mt][:])
                    v_ps[mt] = None

            def y_proj_part(c, part):
                n, k2 = part // 4, 2 * (part % 4)
                if part % 4 == 0:
                    y_ps[c] = psproj.tile([P, T], f32, tag="psproj", name=f"psy_{c}_{n}")
                for m in (k2, k2 + 1):
                    nc.tensor.matmul(y_ps[c][:],
                                     oT_sb[:, m, c * P:(c + 1) * P],
                                     wo_t[m][:, n * T:(n + 1) * T],
                                     start=(part % 4 == 0 and m == k2),
                                     stop=(part % 4 == 3 and m == k2 + 1))
                if part % 4 == 3:
                    y_sb = y_pool.tile([P, T], dtc, tag="ystage")
                    nc.vector.tensor_add(y_sb[:], y_ps[c][:], bo_sb[:, n * T:(n + 1) * T])
                    eng = nc.sync if (c + n) % 2 == 0 else nc.gpsimd
                    eng.dma_start(y_d[c * P:(c + 1) * P, n * T:(n + 1) * T], y_sb[:])
                    y_ps[c] = None

            # ---- V projection for chunk 0 ----
            for part in range(KO):
                v_proj_part(0, part)

            # ---- attention per (chunk, head pair) + interleaved V/Y parts,
            # software-pipelined one super-iteration deep: the rowsum/recip/
            # O^T/mul "back half" of super-iteration s is emitted during
            # s+1, so the PE never waits on s's exp+memset chain.
            def emit_back(bc, bhps, bp2s):
                btsl = slice(bc * P, (bc + 1) * P)
                bpair = (bhps, bhps + 1)
                ro, recs = {}, {}
                for hp in bpair:
                    p2 = bp2s[hp]
                    ps_ro = ro[hp] = psro_pool.tile([P, 2 * P], f32, tag="psro",
                                                    name=f"psro{hp % 2}")
                    nc.tensor.matmul(ps_ro[0:64, 0:P], ones_sb[:],
                                     p2[:, 0:P], start=True, stop=True)
                    nc.tensor.matmul(ps_ro[64:128, 0:P], ones_sb[:],
                                     p2[:, P:2 * P], start=True, stop=True)
                for hp in bpair:
                    rec = recs[hp] = rec_pool.tile([P, P], f32, tag="rec",
                                                   name=f"rec{hp % 2}")
                    nc.vector.reciprocal_approx_fast(out=rec[:], in_=ro[hp][:, 0:P])
                    for idx, h in ((0, 2 * hp), (1, 2 * hp + 1)):
                        nc.tensor.matmul(ro[hp][idx * 64:(idx + 1) * 64, P:2 * P],
                                         v_sb[:, bc, h * DK:(h + 1) * DK],
                                         bp2s[hp][:, idx * P:(idx + 1) * P],
                                         start=True, stop=True)
                for hp in bpair:
                    nc.vector.tensor_mul(oT_sb[:, hp, btsl], ro[hp][:, P:2 * P],
                                         recs[hp][:])

            pending = None
            for c in range(NC_CHUNKS):
                tsl = slice(c * P, (c + 1) * P)
                for hps in range(0, HP, 2):
                    hpair = (hps, hps + 1)
                    # scores: one K=64 matmul per head, each into a full
                    # [128,128] psum tile (K<128 matmuls mis-lower on HW
                    # unless the psum out is a whole tile)
                    ps_s = {}
                    for hp in hpair:
                        ps_s[hp, 0] = pss_pool.tile([P, P], f32, tag="pss", name=f"ps_sa{hp % 2}")
                        ps_s[hp, 1] = pss_pool.tile([P, P], f32, tag="pss", name=f"ps_sb{hp % 2}")
                        nc.tensor.matmul(ps_s[hp, 0][:],
                                         kT_sb[0:64, hp, tsl], qT_sb[0:64, hp, tsl],
                                         start=True, stop=True)
                        nc.tensor.matmul(ps_s[hp, 1][:],
                                         kT_sb[64:128, hp, tsl], qT_sb[64:128, hp, tsl],
                                         start=True, stop=True)
                    # one exp per head (vs 4 quadrant exps -- scalar was the
                    # hidden serializer); gpsimd memsets zero the
                    # exp(garbage) cross-block quadrants afterwards
                    p2s = {}
                    for hp in hpair:
                        p2 = p2s[hp] = p2_pool.tile([P, 2 * P], dtc, tag="p2",
                                                    name=f"p2_{hp % 2}")
                        for idx in (0, 1):
                            nc.scalar.activation(p2[:, idx * P:(idx + 1) * P],
                                                 ps_s[hp, idx][:],
                                                 mybir.ActivationFunctionType.Exp,
                                                 scale=0.125)
                        for o in (0, P):
                            nc.gpsimd.memset(p2[0:64, o + 64:o + 128], 0.0)
                            nc.gpsimd.memset(p2[64:128, o:o + 64], 0.0)
                    # back half of the previous super-iteration
                    if pending is not None:
                        emit_back(*pending)
                    # dense proj matmuls
                    for hp in hpair:
                        if c + 1 < NC_CHUNKS:
                            v_proj_part(c + 1, hp)
                        if c > 0:
                            y_proj_part(c - 1, hp)
                    if c == NC_CHUNKS - 1:
                        # last chunk: everything whose oT deps are already
                        # done runs here, so only m6,m7 trail the last mul
                        if hps == 4:
                            y_proj_part(c, 0)        # n0 m0,1
                            y_proj_part(c, 1)        # n0 m2,3
                        if hps == 6:
                            y_proj_part(c, 2)        # n0 m4,5
                            y_tail[0] = psproj.tile([P, T], f32, tag="psproj",
                                                    name="psy_tail")
                            for m in range(6):       # n1 m0..5
                                nc.tensor.matmul(y_tail[0][:],
                                                 oT_sb[:, m, c * P:(c + 1) * P],
                                                 wo_t[m][:, T:2 * T],
                                                 start=(m == 0), stop=False)
                    pending = (c, hps, p2s)

            emit_back(*pending)
            c = NC_CHUNKS - 1
            y_proj_part(c, 3)                # n0 m6,7 + add + dma
            for m in (6, 7):                 # n1 m6,7, then add + dma
                nc.tensor.matmul(y_tail[0][:],
                                 oT_sb[:, m, c * P:(c + 1) * P],
                                 wo_t[m][:, T:2 * T],
                                 start=False, stop=(m == 7))
            y_sb = y_pool.tile([P, T], dtc, tag="ystage", name="ytail")
            nc.vector.tensor_add(y_sb[:], y_tail[0][:], bo_sb[:, T:2 * T])
            nc.sync.dma_start(y_d[c * P:(c + 1) * P, T:2 * T], y_sb[:])

    nc.compile()
    return nc


def _get_program(compute):
    if compute not in _cache:
        _cache[compute] = _build_program(compute)
    return _cache[compute]


DEFAULT_COMPUTE = "f16"


def kernel(
    query,
    key,
    value,
    Wq,
    bq,
    Wk,
    bk,
    Wv,
    bv,
    Wo,
    bo,
    _compute=DEFAULT_COMPUTE,
    _trace=False,
):
    from concourse.bass_utils import run_bass_kernel_spmd

    nc = _get_program(_compute)
    if _compute == "bf16":
        import ml_dtypes

        npdt = ml_dtypes.bfloat16
    else:
        npdt = {"f32": np.float32, "f16": np.float16}[_compute]

    def pre_w(w):
        # [din, dout] -> [m, p, ko, c] tiles so each m-tile DMAs contiguously
        return np.ascontiguousarray(
            np.asarray(w, np.float32)
            .reshape(KO, P, MO, P)
            .transpose(2, 1, 0, 3)
            .astype(npdt)
        )

    def pre_x(x2, rows):
        # [tok, din] slice -> [p, ko, t] (partition-major, 4KB half-lines)
        return np.ascontiguousarray(
            x2[rows].T.reshape(KO, P, T).transpose(1, 0, 2).astype(npdt)
        )

    q2 = np.asarray(query, np.float32).reshape(B * S, D)
    k2 = np.asarray(key, np.float32).reshape(B * S, D)
    v2 = np.asarray(value, np.float32).reshape(B * S, D)
    # bv folds through attention (rows sum to 1): y = attn Wo + (bv Wo + bo)
    bo_eff = (np.asarray(bv, np.float64) @ np.asarray(Wo, np.float64)
              + np.asarray(bo, np.float64)).astype(np.float32)
    shared = {
        "wq": pre_w(Wq),
        "wk": pre_w(Wk),
        "wv": np.ascontiguousarray(np.asarray(Wv, np.float32).astype(npdt)),
        "wo": np.ascontiguousarray(np.asarray(Wo, np.float32).astype(npdt)),
        "bq": np.ascontiguousarray(np.asarray(bq, np.float32).reshape(MO, P).T),
        "bo": np.ascontiguousarray(bo_eff.astype(npdt)),
    }
    in_maps = []
    for c in range(N_CORES):
        rows = slice(c * T, (c + 1) * T)
        in_maps.append(
            {
                "xq": pre_x(q2, rows),
                "xk": pre_x(k2, rows),
                "xv": pre_x(v2, rows),
                **shared,
            }
        )

    kwargs = {}
    if _trace:
        kwargs = {"trace": True}
    res = run_bass_kernel_spmd(nc, in_maps, core_ids=list(range(N_CORES)), **kwargs)
    y = np.concatenate(
        [res.results[c]["y"].astype(np.float32) for c in range(N_CORES)], axis=0
    )
    out = y.reshape(B, S, D)
    if _trace:
        return out, res
    return out
